# revision 6
# baseline (speedup 1.0000x reference)
"""Trainium2 Bass kernel for DKWinners (overlapping-window k-winners masking).

Problem: x [512, 65536] f32. boosted = x * exp((1/16 - duty_cycle) * bs).
For each of 4096 windows (window k covers boosted cols [15k, 15k+16)),
find the max; the output keeps x[:, 16k+j] where j is the argmax slot of
window k (mask laid on the non-overlapping 16-grid) and zeros the rest.

Sharding (8 cores): 4 batch-quarters x 2 column-halves. Each core handles
128 batch rows (= the 128 SBUF partitions) and 2048 windows (one column
half). Column half h covers boosted cols [30720h, 30720h+30721) and out
cols [32768h, 32768h+32768); the per-core x shard is x[:, 30720h : +34816]
which contains both.

Pipeline (per chunk of wc windows, lb = 15*wc+1 boost cols):
  PE broadcasts the boost-factor slice to all 128 partitions with K=3
  matmuls into PSUM (f split into three bf16 terms whose fp32 sum
  reconstructs it exactly); two PSUM buffers so chunk c+1's broadcast
  overlaps chunk c's reads.
  ACT mirrors the whole PSUM broadcast into SBUF (bit-exact copy, 0.83
  ns/elem) -- the only PSUM reader.
  GPSIMD computes all of b = x*f from the mirror (1.9 ns/elem), then the
  deferred GPS share of the previous chunk's out-multiply, then issues
  that share's output DMA from its own sequencer (25 ns).
  DVE: per-window max M via a 3D strided tensor_reduce, winner mask via
  is_equal against the broadcast max (fp32-exact, so equality == argmax;
  only DVE has is_equal), and the first OUT_DVE_FRAC of the out-multiply
  (1.04 ns/elem). ACT issues the DVE share's output DMA.
This splits the ~150 DVE-equivalent-us of elementwise work so DVE and
GPS both land at ~90us, under the ~85-90us DMA floor (the two HBM
streams sustain ~420 GB/s aggregate). f loads are prefetched 2 chunks
ahead on the stripe queue with 3 buffers so the in-order queue head
never blocks stripe loads; chunks taper at both ends to shorten the
pipeline ramp and drain.
The out grid sits +2048 cols relative to the boost grid on upper-half
cores; that offset is runtime-computed from partition_id so one SPMD
program serves all 8 cores.
"""
import numpy as np
from contextlib import ExitStack

BATCH = 512
N = 65536
OUT_DIM = 4096
DPC = 16
NCORES = 8

H_WINDOWS = 2048          # windows per column half
WCMAX = 128
# chunk plan: (window_start, window_count); tapered at both ends
_sizes = [32, 64, 96] + [128] * 13 + [96, 64, 32]
assert sum(_sizes) == H_WINDOWS
CHUNKS = []
_w = 0
for _s in _sizes:
    CHUNKS.append((_w, _s))
    _w += _s
LBMAX = 15 * WCMAX + 1
OUT_DVE_FRAC = 0.58       # fraction of each chunk's out-multiply on DVE
XS_COLS = 34816           # per-core x shard cols
OUT_COLS = 32768          # per-core out cols
FS_COLS = 15 * H_WINDOWS + 1  # boost-factor cols per half (30721)
# x stripe plan: small leading stripes so chunk 0 starts early
STRIPES = [544, 1632] + [2176] * 15
assert sum(STRIPES) == XS_COLS

_CACHE: dict = {}


def _build():
    import concourse.bacc as bacc
    import concourse.bass as bass
    import concourse.mybir as mybir
    import concourse.tile as tile
    from concourse.ap import AP

    f32 = mybir.dt.float32
    bf16 = mybir.dt.bfloat16

    nc = bacc.Bacc(
        "TRN2", target_bir_lowering=False, debug=False, num_devices=NCORES
    )
    xs = nc.dram_tensor("xs", [128, XS_COLS], f32, kind="ExternalInput")
    fs3 = nc.dram_tensor("fs3", [3, FS_COLS], bf16, kind="ExternalInput")
    out = nc.dram_tensor("out", [128, OUT_COLS], f32, kind="ExternalOutput")

    def win_view(ap, w0, nwin):
        """[128, nwin, 16] overlapping-window view (stride 15), starting at
        window w0 of the buffer AP."""
        base = ap[:, 15 * w0 : 15 * w0 + 15 * nwin + 1]
        return AP(base.tensor, base.offset,
                  [list(base.ap[0]), [15, nwin], [1, DPC]])

    def grid_view(ap, w0, nwin):
        """[128, nwin, 16] contiguous 16-grid view starting at window w0."""
        base = ap[:, 16 * w0 : 16 * (w0 + nwin)]
        return AP(base.tensor, base.offset,
                  [list(base.ap[0]), [DPC, nwin], [1, DPC]])

    with tile.TileContext(nc) as tc, ExitStack() as ctx:
        bpool = ctx.enter_context(tc.tile_pool(name="b", bufs=2))
        opool = ctx.enter_context(tc.tile_pool(name="o", bufs=3))
        fpool = ctx.enter_context(tc.tile_pool(name="f", bufs=3))
        fcpool = ctx.enter_context(tc.tile_pool(name="fc", bufs=2))
        mpool = ctx.enter_context(tc.tile_pool(name="m", bufs=2))
        psum = ctx.enter_context(tc.tile_pool(name="ps", bufs=2, space="PSUM"))

        # Static allocations outside the pools (no pool padding).
        xs_sb = nc.alloc_sbuf_tensor("xs_sb", [128, XS_COLS], f32).ap()
        ones = nc.alloc_sbuf_tensor("ones_sb", [3, 128], bf16).ap()
        nc.vector.memset(ones, 1.0)

        stripe_off = [0]
        for w in STRIPES:
            stripe_off.append(stripe_off[-1] + w)

        def load_stripe(s):
            nc.sync.dma_start(
                xs_sb[:, stripe_off[s] : stripe_off[s + 1]],
                xs[:, stripe_off[s] : stripe_off[s + 1]],
            )

        def load_f(ci):
            w0, wc = CHUNKS[ci]
            lb = 15 * wc + 1
            t = fpool.tile([3, LBMAX], bf16)
            nc.sync.dma_start(t[0:3, 0:lb], fs3[0:3, 15 * w0 : 15 * w0 + lb])
            return t

        # Out-grid column offset: +2048 on upper-half cores (ids 4..7).
        pid = nc.partition_id(
            engines=[mybir.EngineType.DVE, mybir.EngineType.Pool]
        )
        o0 = (pid >= 4) * 2048

        f_tiles = {0: load_f(0), 1: load_f(1), 2: load_f(2)}
        load_stripe(0)
        load_stripe(1)
        next_stripe = 2

        # GPS's share of the out-multiply of chunk c runs after chunk c+1's
        # GPS boost multiply so the DVE reduce never waits behind it; GPS
        # then issues that share's output DMA itself.
        pending = None   # (o_tile, w0, wc, wr)

        def emit_gps_outmul(p):
            o, w0, wc, wr = p
            c0, c1 = 16 * wr, 16 * wc
            xo0 = o0 + 16 * w0
            nc.gpsimd.tensor_tensor(
                o[:, c0:c1], o[:, c0:c1], xs_sb[:, bass.ds(xo0 + c0, c1 - c0)],
                mybir.AluOpType.mult,
            )
            nc.gpsimd.dma_start(
                out[:, 16 * w0 + c0 : 16 * w0 + c1], o[:, c0:c1]
            )

        for ci, (w0, wc) in enumerate(CHUNKS):
            # f loads first so the in-order queue never blocks stripes
            if ci + 2 < len(CHUNKS) and ci + 2 not in f_tiles:
                f_tiles[ci + 2] = load_f(ci + 2)
            # keep stripes one chunk ahead of consumption
            if ci + 1 < len(CHUNKS):
                nw0, nwc = CHUNKS[ci + 1]
                need = 2048 + 16 * (nw0 + nwc)
                target = 0
                while target < len(STRIPES) - 1 and stripe_off[target + 1] < need:
                    target += 1
            else:
                target = len(STRIPES) - 1
            while next_stripe <= target:
                load_stripe(next_stripe)
                next_stripe += 1
            f_sb = f_tiles.pop(ci)

            lb = 15 * wc + 1
            wr = int(round(OUT_DVE_FRAC * wc))

            # PE: broadcast f chunk into PSUM (one K=3 matmul per <=512-col
            # piece sums the three bf16 terms exactly in fp32).
            f_ps = psum.tile([128, LBMAX], f32, tag="fps")
            off = 0
            while off < lb:
                n = min(512, lb - off)
                nc.tensor.matmul(
                    f_ps[:, off : off + n], ones, f_sb[0:3, off : off + n]
                )
                off += n

            # ACT: bit-exact mirror of the whole broadcast into SBUF.
            fc = fcpool.tile([128, LBMAX], f32, tag="fc")
            nc.scalar.copy(fc[:, :lb], f_ps[:, :lb])

            fb = 15 * w0
            b = bpool.tile([128, LBMAX], f32, tag="b")
            # GPS boost multiply first, so b lands before DVE needs it for
            # the reduce; the previous chunk's out-multiply queues behind it.
            nc.gpsimd.tensor_tensor(
                b[:, :lb], xs_sb[:, fb : fb + lb], fc[:, :lb],
                mybir.AluOpType.mult,
            )
            if pending is not None:
                emit_gps_outmul(pending)
                pending = None

            M = mpool.tile([128, WCMAX], f32, tag="m")
            nc.vector.tensor_reduce(
                M[:, :wc], win_view(b, 0, wc), axis=mybir.AxisListType.X,
                op=mybir.AluOpType.max,
            )

            o = opool.tile([128, 16 * WCMAX], f32, tag="o")
            M3 = M[:, :wc].unsqueeze(2).broadcast_to([128, wc, DPC])
            nc.vector.tensor_tensor(
                grid_view(o, 0, wc), win_view(b, 0, wc), M3,
                mybir.AluOpType.is_equal,
            )
            # DVE share of the out-multiply, then its output DMA from ACT.
            xo0 = o0 + 16 * w0
            nc.vector.tensor_tensor(
                o[:, : 16 * wr], o[:, : 16 * wr],
                xs_sb[:, bass.ds(xo0, 16 * wr)],
                mybir.AluOpType.mult,
            )
            nc.scalar.dma_start(
                out[:, 16 * w0 : 16 * (w0 + wr)], o[:, : 16 * wr]
            )
            pending = (o, w0, wc, wr)

        emit_gps_outmul(pending)

    nc.compile()
    return nc


def _get_nc():
    if "nc" not in _CACHE:
        _CACHE["nc"] = _build()
    return _CACHE["nc"]


def _split_bf16_3(f):
    """Split fp32 f into three bf16 terms whose fp32 sum is exactly f
    (verified by assertion, in both association orders)."""
    import ml_dtypes

    bf = ml_dtypes.bfloat16
    hi = f.astype(bf)
    r = (f - hi.astype(np.float32)).astype(np.float32)
    mid = r.astype(bf)
    r2 = (r - mid.astype(np.float32)).astype(np.float32)
    lo = r2.astype(bf)
    f32 = np.float32
    assert np.array_equal((hi.astype(f32) + mid.astype(f32)) + lo.astype(f32), f)
    assert np.array_equal(hi.astype(f32) + (mid.astype(f32) + lo.astype(f32)), f)
    return np.stack([hi, mid, lo], axis=0)


def _shard_inputs(x, duty_cycle, boost_strength):
    x = np.ascontiguousarray(x, dtype=np.float32)
    duty = np.asarray(duty_cycle, dtype=np.float32)
    bs = np.asarray(boost_strength, dtype=np.float32)

    # Boost factors, matching the reference's fp32 arithmetic: the product
    # is computed in fp32 exactly as jax does; exp is evaluated in float64
    # and rounded once to fp32 (correctly-rounded expf).
    t = (np.float32(OUT_DIM / N) - duty) * bs[0]
    f = np.exp(t.astype(np.float64)).astype(np.float32)

    in_maps = []
    for i in range(NCORES):
        q, h = i % 4, i // 4
        xs_i = np.ascontiguousarray(
            x[128 * q : 128 * (q + 1), 30720 * h : 30720 * h + XS_COLS]
        )
        fs_i = _split_bf16_3(
            np.ascontiguousarray(f[30720 * h : 30720 * h + FS_COLS])
        )
        in_maps.append({"xs": xs_i, "fs3": fs_i})
    return in_maps


def _assemble(results):
    full = np.empty((BATCH, N), dtype=np.float32)
    for i in range(NCORES):
        q, h = i % 4, i // 4
        full[
            128 * q : 128 * (q + 1), 32768 * h : 32768 * h + OUT_COLS
        ] = results[i]["out"]
    return full


def kernel(x, duty_cycle, boost_strength):
    from concourse.bass_utils import run_bass_kernel_spmd

    nc = _get_nc()
    in_maps = _shard_inputs(x, duty_cycle, boost_strength)
    res = run_bass_kernel_spmd(nc, in_maps, list(range(NCORES)))
    return _assemble(res.results)
